# revision 24
# baseline (speedup 1.0000x reference)
"""DecoderLSTM Trainium2 kernel (v3).

Computes, for inputs matching the reference nn module:
    x  = embed_table[captions]                      # [B, T, E]
    xg = einsum('bte,ge->tbg', x, W_ih) + b_ih + b_hh
    (h, c) LSTM scan over T steps, h0 = features, c0 = 0
    out = einsum('tbh,vh->btv', hs, W_out) + b_out  # [B, T, V]

Sharding: 8 cores = 4 batch blocks (32 rows) x 2 vocab halves (5000).
Each core runs the LSTM recurrence for its 32 batch rows (duplicated
across the 2 vocab halves -- gate matmul wall time is M-independent up
to M=32 per PE column group, so duplication is free) and projects onto
its 5120-padded vocab half.  b_out is added host-side.

Performance structure:
  - xg = EW[captions] where EW = embed @ W_ih.T + bias is precomputed on
    the host (bf16 [V, 2048], gate-permuted).  Time-block 0 (4 steps) is
    host-gathered and shipped directly; blocks 1-4 are row-gathered on
    device (contiguous 4KB rows, gpsimd ring, nothing else on that ring).
  - The xg inject matmuls read the gathered rows in place (lhsT is a
    [128,32] stacked identity sliced at partition r0=(t%4)*32).
  - Per step the gate matmul is split into three accumulation sets --
    (i,f) N=256 -> PSUM tile A, (g) N=128 and (o) N=128 -> PSUM tile B
    -- so sigmoid(i,f)/tanh(g) start while the PE still streams o.
  - Single chain, tail = sigm/tanh (ACT, bf16 out) -> c update (DVE) ->
    tanh(c) -> h -> PE transpose -> bf16 cast into hsT.
  - The vocab projection is interleaved into the recurrence tail windows
    (slots 1-8 N=256 during steps 8-15, slots 9-16 for v<20 during steps
    16-19; the rest post-recurrence at N=384/128).  Output stored bf16.
  - ~28 warmup dummy matmuls before step 0 keep the PE HAM clock at 2.4
    GHz; wout loads ride the tensor ring interleaved with them.
"""

import numpy as np
import ml_dtypes

import concourse.bass as bass
import concourse.mybir as mybir
import concourse.tile as tile
from concourse import bacc

BF16 = mybir.dt.bfloat16
F32 = mybir.dt.float32
I16 = mybir.dt.int16
AF = mybir.ActivationFunctionType

B, T, E, H, V = 128, 20, 300, 512, 10000
NCORES = 8
BC = 32                 # batch rows per core
NBB = 4                 # batch blocks
VHALF = 5000            # vocab rows per half
NV = 40                 # 128-row vocab tiles per half
VPADH = NV * 128        # 5120
NIDX = BC * T           # 640 xg rows per core
NM = NIDX // 128        # 5 time blocks (4 steps each); block 0 shipped direct
NT = BC * T             # 640 output columns per core
NDUM = 12               # PE warmup dummy matmuls


def _gate_perm():
    """gate-dim order: chunk j = [i_j | f_j | g_j | o_j], blocks of 128."""
    perm = np.empty(4 * H, dtype=np.int64)
    n = 0
    for j in range(4):
        for q in range(4):          # i, f, g, o (PyTorch order)
            for r in range(128):
                perm[n] = q * H + j * 128 + r
                n += 1
    return perm


def build_nc():
    nc = bacc.Bacc("TRN2", target_bir_lowering=False, debug=False)

    # ---- DRAM parameters (per-core shapes) ----
    ew_d = nc.dram_tensor("ew", [V, 2048], BF16, kind="ExternalInput")
    xg0_d = nc.dram_tensor("xg0", [128, 2048], BF16, kind="ExternalInput")
    xg1_d = nc.dram_tensor("xg1", [128, 2048], BF16, kind="ExternalInput")
    idx_d = nc.dram_tensor("idx", [128, NM * 8], I16, kind="ExternalInput")
    whh_d = nc.dram_tensor("whh", [4, 128, 4, 512], BF16, kind="ExternalInput")
    wout_d = nc.dram_tensor("wout", [4, 128, NV, 128], BF16, kind="ExternalInput")
    h0t_d = nc.dram_tensor("h0t", [128, 4, BC], BF16, kind="ExternalInput")
    idf_d = nc.dram_tensor("idf", [128, 128], F32, kind="ExternalInput")
    idfb_d = nc.dram_tensor("idfb", [128, 128], BF16, kind="ExternalInput")
    i32b_d = nc.dram_tensor("i32b", [128, 32], BF16, kind="ExternalInput")
    outT_d = nc.dram_tensor("outT", [128, NV, NT], BF16, kind="ExternalOutput")

    with tile.TileContext(nc) as tc:
        with (
            tc.tile_pool(name="const", bufs=1) as const,
            tc.tile_pool(name="wpool", bufs=1) as wpool,
            tc.tile_pool(name="work", bufs=2) as work,
            tc.tile_pool(name="stage", bufs=4) as stage_p,
            tc.tile_pool(name="hold", bufs=12) as hold_p,
            tc.tile_pool(name="psg", bufs=2, space="PSUM") as ps_gates,
            tc.tile_pool(name="psh", bufs=1, space="PSUM") as ps_ht,
            tc.tile_pool(name="psb", bufs=4, space="PSUM") as ps_big,
        ):
            idx_sb = const.tile([128, NM * 8], I16, tag="idx")
            i32b_sb = const.tile([128, 32], BF16, tag="i32b")
            idf_sb = const.tile([128, 128], F32, tag="idf")
            idfb_sb = const.tile([128, 128], BF16, tag="idfb")

            xg_sb = [wpool.tile([128, 1, 2048], BF16, tag=f"xg{m}", name=f"xg{m}")
                     for m in range(NM)]
            whh_sb = [wpool.tile([128, 4, 512], BF16, tag=f"whh{k}", name=f"whh{k}")
                      for k in range(4)]
            wout_sb = [wpool.tile([128, NV, 128], BF16, tag=f"wo{k}", name=f"wo{k}")
                       for k in range(4)]
            # hs_T: slot s holds h after step s-1 (slot 0 = h0):
            # [128p = H-offset within chunk, slot, chunk k, b]
            hsT = wpool.tile([128, T + 1, 4, BC], BF16, tag="hsT")
            C = wpool.tile([128, 128], F32, tag="C")

            # ---- loads ----
            # sync: idf (dummies) first, then xg block 0, small tensors.
            # scalar: whh k0-k3, then wout k0-k3 (all ready at t0 so the
            # ring keeps emission order; whh descriptors lead).
            # gpsimd: idx then the 4 row-gathers -- nothing else, so the
            # gather descriptors are not stuck behind weight floods.
            nc.sync.dma_start(idx_sb[:], idx_d[:])
            nc.sync.dma_start(idf_sb[:], idf_d[:])
            nc.sync.dma_start(idfb_sb[:], idfb_d[:])
            nc.sync.dma_start(i32b_sb[:], i32b_d[:])
            nc.sync.dma_start(xg_sb[1][:], xg1_d[:])
            nc.sync.dma_start(hsT[:, 0, :, :], h0t_d[:])
            nc.scalar.dma_start(whh_sb[0][:], whh_d[0])
            nc.scalar.dma_start(xg_sb[0][:], xg0_d[:])
            nc.scalar.dma_start(whh_sb[1][:], whh_d[1])
            nc.scalar.dma_start(whh_sb[2][:], whh_d[2])
            nc.scalar.dma_start(whh_sb[3][:], whh_d[3])
            for m in range(2, NM):
                nc.gpsimd.dma_gather(xg_sb[m][:], ew_d[:],
                                     idx_sb[:, m * 8:(m + 1) * 8],
                                     128, 128, 2048)
            for k in range(4):
                nc.scalar.dma_start(wout_sb[k][:], wout_d[k])
            nc.vector.memset(C[:], 0.0)

            # ---- PE warmup (HAM -> K=8/8) ----
            dum = ps_big.tile([128, 2, 256], F32, tag="pp", name="dummy")
            for i in range(NDUM):
                nc.tensor.matmul(dum[:, 0, 0:128], idf_sb[:], idf_sb[:],
                                 start=True, stop=True)

            gate_tiles = {}

            def emit_inject(t):
                gp = ps_gates.tile([128, 4, 128], F32, tag="g", name=f"g{t}")
                gate_tiles[t] = gp
                m, r0 = t // 4, (t % 4) * 32
                for j in range(4):
                    nc.tensor.matmul(
                        gp[32 * j:32 * j + 32, :, :],
                        i32b_sb[r0:r0 + 32, :],
                        xg_sb[m][r0:r0 + 32, 0, j * 512:(j + 1) * 512],
                        start=True, stop=False,
                        tile_position=(r0, 32 * j), skip_group_check=True)

            def emit_gates(t):
                gp = gate_tiles[t]
                for k in range(4):
                    for j in range(4):
                        nc.tensor.matmul(
                            gp[32 * j:32 * j + 32, :, :],
                            hsT[:, t, k, :],
                            whh_sb[k][:, j, :],
                            start=False, stop=(k == 3),
                            tile_position=(0, 32 * j), skip_group_check=True)

            def emit_tail_pre(t):
                """sigm/tanh + c/h update, up to Hn (bf16)."""
                gp = gate_tiles.pop(t)
                A = work.tile([128, 2, 128], F32, tag="A", name=f"A{t}")
                TG = work.tile([128, 128], F32, tag="TG", name=f"TG{t}")
                AO = work.tile([128, 128], BF16, tag="AO", name=f"AO{t}")
                TC = work.tile([128, 128], BF16, tag="TC", name=f"TC{t}")
                T1 = work.tile([128, 128], F32, tag="T1", name=f"T1{t}")
                T2 = work.tile([128, 128], F32, tag="T2", name=f"T2{t}")
                Hn = work.tile([128, 128], BF16, tag="Hn", name=f"Hn{t}")
                nc.scalar.activation(A[:], gp[:, 0:2, :], AF.Sigmoid)
                nc.scalar.activation(TG[:], gp[:, 2, :], AF.Tanh)
                nc.scalar.activation(AO[:], gp[:, 3, :], AF.Sigmoid)
                nc.vector.tensor_mul(T1[:], A[:, 1, :], C[:])       # f*c
                nc.vector.tensor_mul(T2[:], A[:, 0, :], TG[:])      # i*tanh(g)
                nc.vector.tensor_add(C[:], T1[:], T2[:])
                nc.scalar.activation(TC[:], C[:], AF.Tanh)
                nc.vector.tensor_mul(Hn[:], AO[:], TC[:])           # o*tanh(c)
                return Hn

            def emit_transpose(t, Hn):
                hp = ps_ht.tile([128, 4, 32], BF16, tag="ht", name=f"ht{t}")
                nc.tensor.transpose(hp[:], Hn[:], idfb_sb[:])
                nc.vector.tensor_copy(hsT[:, t + 1, :, :], hp[:])

            # ---- projection helpers (b_out added host-side) ----
            evac_flip = [0]

            def emit_proj_pair(v, s0, s1):
                n = (s1 - s0) * BC
                pp = ps_big.tile([128, 2, n], F32, tag="pp", name=f"pp{v}_{s0}")
                for vv in range(2):
                    for k in range(4):
                        nc.tensor.matmul(
                            pp[:, vv, :],
                            wout_sb[k][:, v + vv, :],
                            hsT[:, s0:s1, k, :],
                            start=(k == 0), stop=(k == 3),
                        )
                return pp

            held = []

            def emit_proj_evac(v, s0, s1, pp, hold=False):
                n = (s1 - s0) * BC
                pool = hold_p if hold else stage_p
                st = pool.tile([128, 2, n], BF16, tag="hst" if hold else "st",
                               name=f"st{v}_{s0}")
                if evac_flip[0] == 0:
                    nc.vector.tensor_copy(st[:], pp[:])
                else:
                    nc.scalar.activation(st[:], pp[:], AF.Identity)
                evac_flip[0] ^= 1
                c0 = (s0 - 1) * BC
                if hold:
                    held.append((v, c0, n, st))
                else:
                    nc.sync.dma_start(outT_d[:, v:v + 2, c0:c0 + n], st[:])

            # fill schedule: step -> list of (v, s0, s1) pairs
            fills = {t: [] for t in range(T)}
            q = [(v, 1, 9) for v in range(0, NV, 2)]            # 20 pairs
            q += [(v, 9, 17) for v in range(0, NV // 2, 2)]     # 10 pairs
            counts = {8: 2, 9: 3, 10: 2, 11: 3, 12: 2, 13: 3, 14: 2, 15: 3,
                      16: 2, 17: 3, 18: 2, 19: 3}
            for t in range(T):
                for _ in range(counts.get(t, 0)):
                    if q:
                        fills[t].append(q.pop(0))

            # ---- recurrence ----
            emit_inject(0)
            for t in range(T):
                emit_gates(t)
                Hn = emit_tail_pre(t)
                filled = []
                for (v, s0, s1) in fills[t]:
                    filled.append((v, s0, s1, emit_proj_pair(v, s0, s1)))
                if t + 1 < T:
                    emit_inject(t + 1)
                emit_transpose(t, Hn)
                for item in filled:
                    emit_proj_evac(*item)

            # ---- projection tail ----
            # v 20..39: slots 9-20 in one N=384 sweep (single v-tiles)
            pend = []
            for v in range(NV // 2, NV):
                pp = ps_big.tile([128, 1, 384], F32, tag="pp", name=f"ppt{v}")
                for k in range(4):
                    nc.tensor.matmul(pp[:, 0, :], wout_sb[k][:, v, :],
                                     hsT[:, 9:21, k, :],
                                     start=(k == 0), stop=(k == 3))
                pend.append((v, pp))
                if len(pend) >= 2:
                    for (vv, q2) in pend:
                        st = stage_p.tile([128, 1, 384], BF16, tag="st",
                                          name=f"stt{vv}")
                        if evac_flip[0] == 0:
                            nc.vector.tensor_copy(st[:], q2[:])
                        else:
                            nc.scalar.activation(st[:], q2[:], AF.Identity)
                        evac_flip[0] ^= 1
                        nc.sync.dma_start(outT_d[:, vv:vv + 1, 256:640], st[:])
                    pend = []
            # v 0..19: slots 17-20 (N=128) in pairs
            def emit_c_evac(v, pp):
                st = stage_p.tile([128, 2, 128], BF16, tag="st",
                                  name=f"stc{v}")
                nc.vector.tensor_copy(st[:], pp[:])
                nc.scalar.dma_start(outT_d[:, v:v + 2, 512:640], st[:])

            pend2 = []
            for v in range(0, NV // 2, 2):
                pp = emit_proj_pair(v, 17, 21)
                pend2.append((v, pp))
                if len(pend2) >= 2:
                    for (vv, q2) in pend2:
                        emit_c_evac(vv, q2)
                    pend2 = []
            for (vv, q2) in pend2:
                emit_c_evac(vv, q2)

    nc.compile()
    return nc


def prep_inputs(features, captions, embed_table, W_ih, W_hh, b_ih, b_hh,
                W_out, b_out):
    """Host-side shard + layout prep. Returns per-core input maps."""
    bf = ml_dtypes.bfloat16
    features = np.asarray(features, dtype=np.float32)
    captions = np.asarray(captions).astype(np.int64)
    embed_table = np.asarray(embed_table, dtype=np.float32)
    W_ih = np.asarray(W_ih, dtype=np.float32)
    W_hh = np.asarray(W_hh, dtype=np.float32)
    b_ih = np.asarray(b_ih, dtype=np.float32)
    b_hh = np.asarray(b_hh, dtype=np.float32)
    W_out = np.asarray(W_out, dtype=np.float32)
    b_out = np.asarray(b_out, dtype=np.float32)

    perm = _gate_perm()

    # fused embedding: EW[v] = embed[v] @ W_ih.T + b_ih + b_hh, perm'd
    ew = (embed_table @ W_ih.T + (b_ih + b_hh))[:, perm]
    ew = np.ascontiguousarray(ew).astype(bf)

    whh = np.ascontiguousarray(W_hh.T[:, perm]).astype(bf).reshape(4, 128, 4, 512)

    idf = np.eye(128, dtype=np.float32)
    idfb = np.eye(128, dtype=np.float32).astype(bf)
    i32b = np.tile(np.eye(32, dtype=np.float32), (4, 1)).astype(bf)

    # vocab halves
    wout_h = []
    for vh in range(2):
        wt = np.zeros((H, VPADH), dtype=np.float32)
        wt[:, :VHALF] = W_out.T[:, vh * VHALF:(vh + 1) * VHALF]
        wout_h.append(wt.astype(bf).reshape(4, 128, NV, 128))

    # batch blocks: xg block 0 (host-gathered), gather indices, h0_T
    xg0_b, xg1_b, idx_b, h0t_b = [], [], [], []
    for bb in range(NBB):
        cap_c = captions[bb * BC:(bb + 1) * BC]              # [32, 20]
        row_of = lambda i, m: cap_c[i % 32, m * 4 + i // 32]
        xg0_b.append(np.ascontiguousarray(
            ew[[row_of(i, 0) for i in range(128)]]))         # [128, 2048]
        xg1_b.append(np.ascontiguousarray(
            ew[[row_of(i, 1) for i in range(128)]]))
        idx = np.zeros((128, NM * 8), dtype=np.int16)
        for m in range(NM):
            flat = np.array([row_of(i, m) for i in range(128)], dtype=np.int16)
            blk = flat.reshape(8, 16).T
            idx[:, m * 8:(m + 1) * 8] = np.tile(blk, (8, 1))
        idx_b.append(idx)
        feat_c = features[bb * BC:(bb + 1) * BC]             # [32, 512]
        h0t_b.append(np.ascontiguousarray(
            feat_c.reshape(BC, 4, 128).transpose(2, 1, 0)).astype(bf))

    shared = dict(ew=ew, whh=whh, idf=idf, idfb=idfb, i32b=i32b)
    in_maps = []
    for c in range(NCORES):
        bb, vh = c % NBB, c // NBB
        in_maps.append(dict(shared, xg0=xg0_b[bb], xg1=xg1_b[bb],
                            idx=idx_b[bb], h0t=h0t_b[bb], wout=wout_h[vh]))
    return in_maps


def unshard(core_outs, b_out=None):
    """core_outs: list of 8 arrays [128, NV, 640] bf16 -> full [B, T, V]."""
    full = np.empty((B, T, V), dtype=np.float32)
    for c in range(NCORES):
        bb, vh = c % NBB, c // NBB
        o = np.asarray(core_outs[c]).astype(np.float32)      # [128, 40, 640]
        o = o.transpose(1, 0, 2).reshape(VPADH, NT)[:VHALF]  # [5000, 640]
        o = o.reshape(VHALF, T, BC).transpose(2, 1, 0)       # [32, T, 5000]
        full[bb * BC:(bb + 1) * BC, :, vh * VHALF:(vh + 1) * VHALF] = o
    if b_out is not None:
        full += np.asarray(b_out, dtype=np.float32)[None, None, :]
    return full


_NC_CACHE = {}


def kernel(**inputs) -> np.ndarray:
    from concourse.bass_utils import run_bass_kernel_spmd

    if "nc" not in _NC_CACHE:
        _NC_CACHE["nc"] = build_nc()
    nc = _NC_CACHE["nc"]

    in_maps = prep_inputs(**inputs)
    res = run_bass_kernel_spmd(nc, in_maps, core_ids=list(range(NCORES)))
    return unshard([res.results[c]["outT"] for c in range(NCORES)],
                   b_out=inputs["b_out"])


# revision 25
# speedup vs baseline: 1.2413x; 1.2413x over previous
"""DecoderLSTM Trainium2 kernel (v3).

Computes, for inputs matching the reference nn module:
    x  = embed_table[captions]                      # [B, T, E]
    xg = einsum('bte,ge->tbg', x, W_ih) + b_ih + b_hh
    (h, c) LSTM scan over T steps, h0 = features, c0 = 0
    out = einsum('tbh,vh->btv', hs, W_out) + b_out  # [B, T, V]

Sharding: 8 cores = 4 batch blocks (32 rows) x 2 vocab halves (5000).
Each core runs the LSTM recurrence for its 32 batch rows (duplicated
across the 2 vocab halves -- gate matmul wall time is M-independent up
to M=32 per PE column group, so duplication is free) and projects onto
its 5120-padded vocab half.  b_out is added host-side.

Performance structure:
  - xg = EW[captions] where EW = embed @ W_ih.T + bias is precomputed on
    the host (bf16 [V, 2048], gate-permuted).  Time-block 0 (4 steps) is
    host-gathered and shipped directly; blocks 1-4 are row-gathered on
    device (contiguous 4KB rows, gpsimd ring, nothing else on that ring).
  - The xg inject matmuls read the gathered rows in place (lhsT is a
    [128,32] stacked identity sliced at partition r0=(t%4)*32).
  - Per step the gate matmul is split into three accumulation sets --
    (i,f) N=256 -> PSUM tile A, (g) N=128 and (o) N=128 -> PSUM tile B
    -- so sigmoid(i,f)/tanh(g) start while the PE still streams o.
  - Single chain, tail = sigm/tanh (ACT, bf16 out) -> c update (DVE) ->
    tanh(c) -> h -> PE transpose -> bf16 cast into hsT.
  - The vocab projection is interleaved into the recurrence tail windows
    (slots 1-8 N=256 during steps 8-15, slots 9-16 for v<20 during steps
    16-19; the rest post-recurrence at N=384/128).  Output stored bf16.
  - ~28 warmup dummy matmuls before step 0 keep the PE HAM clock at 2.4
    GHz; wout loads ride the tensor ring interleaved with them.
"""

import numpy as np
import ml_dtypes

import concourse.bass as bass
import concourse.mybir as mybir
import concourse.tile as tile
from concourse import bacc

BF16 = mybir.dt.bfloat16
F32 = mybir.dt.float32
I16 = mybir.dt.int16
AF = mybir.ActivationFunctionType

B, T, E, H, V = 128, 20, 300, 512, 10000
NCORES = 8
BC = 32                 # batch rows per core
NBB = 4                 # batch blocks
VHALF = 5000            # vocab rows per half
NV = 40                 # 128-row vocab tiles per half
VPADH = NV * 128        # 5120
NIDX = BC * T           # 640 xg rows per core
NM = NIDX // 128        # 5 time blocks (4 steps each); block 0 shipped direct
NT = BC * T             # 640 output columns per core
NDUM = 12               # PE warmup dummy matmuls


def _gate_perm():
    """gate-dim order: chunk j = [i_j | f_j | g_j | o_j], blocks of 128."""
    perm = np.empty(4 * H, dtype=np.int64)
    n = 0
    for j in range(4):
        for q in range(4):          # i, f, g, o (PyTorch order)
            for r in range(128):
                perm[n] = q * H + j * 128 + r
                n += 1
    return perm


def build_nc():
    nc = bacc.Bacc("TRN2", target_bir_lowering=False, debug=False)

    # ---- DRAM parameters (per-core shapes) ----
    ew_d = nc.dram_tensor("ew", [V, 2048], BF16, kind="ExternalInput")
    xg0_d = nc.dram_tensor("xg0", [128, 2048], BF16, kind="ExternalInput")
    xg1_d = nc.dram_tensor("xg1", [128, 2048], BF16, kind="ExternalInput")
    idx_d = nc.dram_tensor("idx", [128, NM * 8], I16, kind="ExternalInput")
    whh_d = nc.dram_tensor("whh", [4, 128, 4, 512], BF16, kind="ExternalInput")
    wout_d = nc.dram_tensor("wout", [4, 128, NV, 128], BF16, kind="ExternalInput")
    h0t_d = nc.dram_tensor("h0t", [128, 4, BC], BF16, kind="ExternalInput")
    idf_d = nc.dram_tensor("idf", [128, 128], F32, kind="ExternalInput")
    idfb_d = nc.dram_tensor("idfb", [128, 128], BF16, kind="ExternalInput")
    i32b_d = nc.dram_tensor("i32b", [128, 32], BF16, kind="ExternalInput")
    outT_d = nc.dram_tensor("outT", [128, NV, NT], BF16, kind="ExternalOutput")

    with tile.TileContext(nc) as tc:
        with (
            tc.tile_pool(name="const", bufs=1) as const,
            tc.tile_pool(name="wpool", bufs=1) as wpool,
            tc.tile_pool(name="work", bufs=2) as work,
            tc.tile_pool(name="stage", bufs=4) as stage_p,
            tc.tile_pool(name="hold", bufs=12) as hold_p,
            tc.tile_pool(name="psg", bufs=2, space="PSUM") as ps_gates,
            tc.tile_pool(name="psh", bufs=1, space="PSUM") as ps_ht,
            tc.tile_pool(name="psb", bufs=4, space="PSUM") as ps_big,
        ):
            idx_sb = const.tile([128, NM * 8], I16, tag="idx")
            i32b_sb = const.tile([128, 32], BF16, tag="i32b")
            idf_sb = const.tile([128, 128], F32, tag="idf")
            idfb_sb = const.tile([128, 128], BF16, tag="idfb")

            xg_sb = [wpool.tile([128, 1, 2048], BF16, tag=f"xg{m}", name=f"xg{m}")
                     for m in range(NM)]
            whh_sb = [wpool.tile([128, 4, 512], BF16, tag=f"whh{k}", name=f"whh{k}")
                      for k in range(4)]
            wout_sb = [wpool.tile([128, NV, 128], BF16, tag=f"wo{k}", name=f"wo{k}")
                       for k in range(4)]
            # hs_T: slot s holds h after step s-1 (slot 0 = h0):
            # [128p = H-offset within chunk, slot, chunk k, b]
            hsT = wpool.tile([128, T + 1, 4, BC], BF16, tag="hsT")
            C = wpool.tile([128, 128], F32, tag="C")

            # ---- loads ----
            # sync: idf (dummies) first, then xg block 0, small tensors.
            # scalar: whh k0-k3, then wout k0-k3 (all ready at t0 so the
            # ring keeps emission order; whh descriptors lead).
            # gpsimd: idx then the 4 row-gathers -- nothing else, so the
            # gather descriptors are not stuck behind weight floods.
            nc.sync.dma_start(idx_sb[:], idx_d[:])
            nc.sync.dma_start(idf_sb[:], idf_d[:])
            nc.sync.dma_start(idfb_sb[:], idfb_d[:])
            nc.sync.dma_start(i32b_sb[:], i32b_d[:])
            nc.sync.dma_start(xg_sb[1][:], xg1_d[:])
            nc.sync.dma_start(hsT[:, 0, :, :], h0t_d[:])
            nc.scalar.dma_start(whh_sb[0][:], whh_d[0])
            nc.scalar.dma_start(xg_sb[0][:], xg0_d[:])
            nc.scalar.dma_start(whh_sb[1][:], whh_d[1])
            nc.scalar.dma_start(whh_sb[2][:], whh_d[2])
            nc.scalar.dma_start(whh_sb[3][:], whh_d[3])
            for m in range(2, NM):
                nc.gpsimd.dma_gather(xg_sb[m][:], ew_d[:],
                                     idx_sb[:, m * 8:(m + 1) * 8],
                                     128, 128, 2048)
            for k in range(4):
                nc.scalar.dma_start(wout_sb[k][:], wout_d[k])
            nc.vector.memset(C[:], 0.0)

            # ---- PE warmup (HAM -> K=8/8) ----
            dum = ps_big.tile([128, 2, 256], F32, tag="pp", name="dummy")
            for i in range(NDUM):
                nc.tensor.matmul(dum[:, 0, 0:128], idf_sb[:], idf_sb[:],
                                 start=True, stop=True)

            gate_tiles = {}

            def emit_inject(t):
                gp = ps_gates.tile([128, 4, 128], F32, tag="g", name=f"g{t}")
                gate_tiles[t] = gp
                m, r0 = t // 4, (t % 4) * 32
                for j in range(4):
                    nc.tensor.matmul(
                        gp[32 * j:32 * j + 32, :, :],
                        i32b_sb[r0:r0 + 32, :],
                        xg_sb[m][r0:r0 + 32, 0, j * 512:(j + 1) * 512],
                        start=True, stop=False,
                        tile_position=(r0, 32 * j), skip_group_check=True)

            def emit_gates(t):
                gp = gate_tiles[t]
                for k in range(4):
                    for j in range(4):
                        nc.tensor.matmul(
                            gp[32 * j:32 * j + 32, :, :],
                            hsT[:, t, k, :],
                            whh_sb[k][:, j, :],
                            start=False, stop=(k == 3),
                            tile_position=(0, 32 * j), skip_group_check=True)

            def emit_tail_pre(t):
                """sigm/tanh + c/h update, up to Hn (bf16)."""
                gp = gate_tiles.pop(t)
                A = work.tile([128, 2, 128], F32, tag="A", name=f"A{t}")
                TG = work.tile([128, 128], F32, tag="TG", name=f"TG{t}")
                AO = work.tile([128, 128], BF16, tag="AO", name=f"AO{t}")
                TC = work.tile([128, 128], BF16, tag="TC", name=f"TC{t}")
                T1 = work.tile([128, 128], F32, tag="T1", name=f"T1{t}")
                T2 = work.tile([128, 128], F32, tag="T2", name=f"T2{t}")
                Hn = work.tile([128, 128], BF16, tag="Hn", name=f"Hn{t}")
                nc.scalar.activation(A[:], gp[:, 0:2, :], AF.Sigmoid)
                nc.scalar.activation(TG[:], gp[:, 2, :], AF.Tanh)
                nc.scalar.activation(AO[:], gp[:, 3, :], AF.Sigmoid)
                nc.vector.tensor_mul(T1[:], A[:, 1, :], C[:])       # f*c
                nc.vector.tensor_mul(T2[:], A[:, 0, :], TG[:])      # i*tanh(g)
                nc.vector.tensor_add(C[:], T1[:], T2[:])
                nc.scalar.activation(TC[:], C[:], AF.Tanh)
                nc.vector.tensor_mul(Hn[:], AO[:], TC[:])           # o*tanh(c)
                return Hn

            def emit_transpose(t, Hn):
                hp = ps_ht.tile([128, 4, 32], BF16, tag="ht", name=f"ht{t}")
                nc.tensor.transpose(hp[:], Hn[:], idfb_sb[:])
                nc.vector.tensor_copy(hsT[:, t + 1, :, :], hp[:])

            # ---- projection helpers (b_out added host-side) ----
            evac_flip = [0]

            def emit_proj_pair(v, s0, s1):
                n = (s1 - s0) * BC
                pp = ps_big.tile([128, 2, n], F32, tag="pp", name=f"pp{v}_{s0}")
                for vv in range(2):
                    for k in range(4):
                        nc.tensor.matmul(
                            pp[:, vv, :],
                            wout_sb[k][:, v + vv, :],
                            hsT[:, s0:s1, k, :],
                            start=(k == 0), stop=(k == 3),
                        )
                return pp

            held = []

            def emit_proj_evac(v, s0, s1, pp, hold=False):
                n = (s1 - s0) * BC
                pool = hold_p if hold else stage_p
                st = pool.tile([128, 2, n], BF16, tag="hst" if hold else "st",
                               name=f"st{v}_{s0}")
                if evac_flip[0] == 0:
                    nc.vector.tensor_copy(st[:], pp[:])
                else:
                    nc.scalar.activation(st[:], pp[:], AF.Identity)
                evac_flip[0] ^= 1
                c0 = (s0 - 1) * BC
                if hold:
                    held.append((v, c0, n, st))
                else:
                    nc.sync.dma_start(outT_d[:, v:v + 2, c0:c0 + n], st[:])

            # fill schedule: step -> list of (v, s0, s1) pairs
            fills = {t: [] for t in range(T)}
            q = [(v, 1, 9) for v in range(0, NV, 2)]            # 20 pairs
            q += [(v, 9, 17) for v in range(0, NV // 2, 2)]     # 10 pairs
            counts = {8: 2, 9: 3, 10: 2, 11: 3, 12: 2, 13: 3, 14: 2, 15: 3,
                      16: 2, 17: 3, 18: 2, 19: 3}
            for t in range(T):
                for _ in range(counts.get(t, 0)):
                    if q:
                        fills[t].append(q.pop(0))

            # ---- recurrence ----
            emit_inject(0)
            for t in range(T):
                emit_gates(t)
                if not fills[t] and t > 0:
                    # HAM keep-warm: tied to this step's hsT slot so the
                    # scheduler cannot hoist them ahead of the recurrence
                    for _ in range(4):
                        nc.tensor.matmul(dum[0:32, :, :], hsT[:, t, 0, :],
                                         whh_sb[0][:, 0, :],
                                         start=True, stop=True)
                Hn = emit_tail_pre(t)
                filled = []
                for (v, s0, s1) in fills[t]:
                    filled.append((v, s0, s1, emit_proj_pair(v, s0, s1)))
                if t + 1 < T:
                    emit_inject(t + 1)
                emit_transpose(t, Hn)
                for item in filled:
                    emit_proj_evac(*item)

            # ---- projection tail ----
            # v 20..39: slots 9-20 in one N=384 sweep (single v-tiles)
            pend = []
            for v in range(NV // 2, NV):
                pp = ps_big.tile([128, 1, 384], F32, tag="pp", name=f"ppt{v}")
                for k in range(4):
                    nc.tensor.matmul(pp[:, 0, :], wout_sb[k][:, v, :],
                                     hsT[:, 9:21, k, :],
                                     start=(k == 0), stop=(k == 3))
                pend.append((v, pp))
                if len(pend) >= 2:
                    for (vv, q2) in pend:
                        st = stage_p.tile([128, 1, 384], BF16, tag="st",
                                          name=f"stt{vv}")
                        if evac_flip[0] == 0:
                            nc.vector.tensor_copy(st[:], q2[:])
                        else:
                            nc.scalar.activation(st[:], q2[:], AF.Identity)
                        evac_flip[0] ^= 1
                        nc.sync.dma_start(outT_d[:, vv:vv + 1, 256:640], st[:])
                    pend = []
            # v 0..19: slots 17-20 (N=128) in pairs
            def emit_c_evac(v, pp):
                st = stage_p.tile([128, 2, 128], BF16, tag="st",
                                  name=f"stc{v}")
                if evac_flip[0] == 0:
                    nc.vector.tensor_copy(st[:], pp[:])
                else:
                    nc.scalar.activation(st[:], pp[:], AF.Identity)
                evac_flip[0] ^= 1
                nc.sync.dma_start(outT_d[:, v:v + 2, 512:640], st[:])

            pend2 = []
            for v in range(0, NV // 2, 2):
                pp = emit_proj_pair(v, 17, 21)
                pend2.append((v, pp))
                if len(pend2) >= 2:
                    for (vv, q2) in pend2:
                        emit_c_evac(vv, q2)
                    pend2 = []
            for (vv, q2) in pend2:
                emit_c_evac(vv, q2)

    nc.compile()
    return nc


def prep_inputs(features, captions, embed_table, W_ih, W_hh, b_ih, b_hh,
                W_out, b_out):
    """Host-side shard + layout prep. Returns per-core input maps."""
    bf = ml_dtypes.bfloat16
    features = np.asarray(features, dtype=np.float32)
    captions = np.asarray(captions).astype(np.int64)
    embed_table = np.asarray(embed_table, dtype=np.float32)
    W_ih = np.asarray(W_ih, dtype=np.float32)
    W_hh = np.asarray(W_hh, dtype=np.float32)
    b_ih = np.asarray(b_ih, dtype=np.float32)
    b_hh = np.asarray(b_hh, dtype=np.float32)
    W_out = np.asarray(W_out, dtype=np.float32)
    b_out = np.asarray(b_out, dtype=np.float32)

    perm = _gate_perm()

    # fused embedding: EW[v] = embed[v] @ W_ih.T + b_ih + b_hh, perm'd
    ew = (embed_table @ W_ih.T + (b_ih + b_hh))[:, perm]
    ew = np.ascontiguousarray(ew).astype(bf)

    whh = np.ascontiguousarray(W_hh.T[:, perm]).astype(bf).reshape(4, 128, 4, 512)

    idf = np.eye(128, dtype=np.float32)
    idfb = np.eye(128, dtype=np.float32).astype(bf)
    i32b = np.tile(np.eye(32, dtype=np.float32), (4, 1)).astype(bf)

    # vocab halves
    wout_h = []
    for vh in range(2):
        wt = np.zeros((H, VPADH), dtype=np.float32)
        wt[:, :VHALF] = W_out.T[:, vh * VHALF:(vh + 1) * VHALF]
        wout_h.append(wt.astype(bf).reshape(4, 128, NV, 128))

    # batch blocks: xg block 0 (host-gathered), gather indices, h0_T
    xg0_b, xg1_b, idx_b, h0t_b = [], [], [], []
    for bb in range(NBB):
        cap_c = captions[bb * BC:(bb + 1) * BC]              # [32, 20]
        row_of = lambda i, m: cap_c[i % 32, m * 4 + i // 32]
        xg0_b.append(np.ascontiguousarray(
            ew[[row_of(i, 0) for i in range(128)]]))         # [128, 2048]
        xg1_b.append(np.ascontiguousarray(
            ew[[row_of(i, 1) for i in range(128)]]))
        idx = np.zeros((128, NM * 8), dtype=np.int16)
        for m in range(NM):
            flat = np.array([row_of(i, m) for i in range(128)], dtype=np.int16)
            blk = flat.reshape(8, 16).T
            idx[:, m * 8:(m + 1) * 8] = np.tile(blk, (8, 1))
        idx_b.append(idx)
        feat_c = features[bb * BC:(bb + 1) * BC]             # [32, 512]
        h0t_b.append(np.ascontiguousarray(
            feat_c.reshape(BC, 4, 128).transpose(2, 1, 0)).astype(bf))

    shared = dict(ew=ew, whh=whh, idf=idf, idfb=idfb, i32b=i32b)
    in_maps = []
    for c in range(NCORES):
        bb, vh = c % NBB, c // NBB
        in_maps.append(dict(shared, xg0=xg0_b[bb], xg1=xg1_b[bb],
                            idx=idx_b[bb], h0t=h0t_b[bb], wout=wout_h[vh]))
    return in_maps


def unshard(core_outs, b_out=None):
    """core_outs: list of 8 arrays [128, NV, 640] bf16 -> full [B, T, V]."""
    full = np.empty((B, T, V), dtype=np.float32)
    for c in range(NCORES):
        bb, vh = c % NBB, c // NBB
        o = np.asarray(core_outs[c]).astype(np.float32)      # [128, 40, 640]
        o = o.transpose(1, 0, 2).reshape(VPADH, NT)[:VHALF]  # [5000, 640]
        o = o.reshape(VHALF, T, BC).transpose(2, 1, 0)       # [32, T, 5000]
        full[bb * BC:(bb + 1) * BC, :, vh * VHALF:(vh + 1) * VHALF] = o
    if b_out is not None:
        full += np.asarray(b_out, dtype=np.float32)[None, None, :]
    return full


_NC_CACHE = {}


def kernel(**inputs) -> np.ndarray:
    from concourse.bass_utils import run_bass_kernel_spmd

    if "nc" not in _NC_CACHE:
        _NC_CACHE["nc"] = build_nc()
    nc = _NC_CACHE["nc"]

    in_maps = prep_inputs(**inputs)
    res = run_bass_kernel_spmd(nc, in_maps, core_ids=list(range(NCORES)))
    return unshard([res.results[c]["outT"] for c in range(NCORES)],
                   b_out=inputs["b_out"])
